# revision 4
# baseline (speedup 1.0000x reference)
"""CenterLoss Trainium2 kernel — raw Bacc, single dma_gather (v5).

Per core (512 samples), bf16 payloads (rel tol is 2e-2; bf16 noise on
2M summed squared terms averages out to ~1e-3):

  scalar(ACT): x DMA (hoisted into entry, HWDGE, parallel with lab);
               warm-up Square (hoists act-table load); one big
               Square+accum over [128, 4, 256] -> d[128,1] fp32;
               out DMA of d (same engine, no cross-engine hop)
  sync(SP)   : idx DMA (hoisted into entry, HWDGE)
  gpsimd     : ONE InstDMAGatherAnt for all 512 rows (994ns SWDGE
               fixed cost paid once instead of 4x)
  vector     : one big tensor_sub [128, 4, 256]

dma_gather semantics (non-transpose): gathered row j = cen[idx_j]
lands at out[j % 128, j // 128, :], with idx_j read from
idxs[j % 16, j // 16] (int16, first 16 partitions of a [128,
num_idxs/16] AP). So with sample(p, n) = n*128 + p:
  x_t[p, n, :] = x[n*128 + p, :]        (host: reshape+transpose)
  c_t[p, n, :] = centers[labels[n*128 + p]]
  idx[r, c]    = labels[c*16 + r]       (host: reshape(32,16).T)
  d[p, 0] = sum_n ||x_t - c_t||^2       (ACT Square accumulator)
partial = d[128, 1] per core; host sums across partitions and cores.
"""

import sys

import numpy as np

if "/opt/trn_rl_repo" not in sys.path:
    sys.path.insert(0, "/opt/trn_rl_repo")

import ml_dtypes

B = 4096
D = 256
C = 8192
M = 8
SHARD = B // M   # 512
P = 128
NT = SHARD // P  # 4
NIC = SHARD // 16  # idx columns: 32

_CACHE = {}


def build_nc():
    import concourse.bacc as bacc
    import concourse.mybir as mybir

    f32 = mybir.dt.float32
    bf16 = mybir.dt.bfloat16
    i16 = mybir.dt.int16

    nc = bacc.Bacc("TRN2", dynamic_dma_scratch_size=32768)
    x = nc.dram_tensor("x", [P, NT, D], bf16, kind="ExternalInput")
    idx = nc.dram_tensor("idx", [P, NIC], i16, kind="ExternalInput")
    cen = nc.dram_tensor("cen", [C, D], bf16, kind="ExternalInput")
    out = nc.dram_tensor("out", [P, 1], f32, kind="ExternalOutput")

    ones_bf = nc.const_aps.aps[(bf16, 1.0)]  # [128, 1] const, preamble memset

    with (
        nc.sbuf_tensor("x_t", [P, NT, D], bf16) as x_t,
        nc.sbuf_tensor("c_t", [P, NT, D], bf16) as c_t,
        nc.sbuf_tensor("diff", [P, NT, D], bf16) as diff,
        nc.sbuf_tensor("sq", [P, NT, D], bf16) as sq,
        nc.sbuf_tensor("idx_t", [P, NIC], i16) as idx_t,
        nc.sbuf_tensor("d", [P, 1], f32) as d,
        nc.sbuf_tensor("warm", [1, 1], bf16) as warm,
        nc.semaphore("i_s") as i_s,
        nc.semaphore("x_s") as x_s,
        nc.semaphore("g_s") as g_s,
        nc.semaphore("v_s") as v_s,
        nc.semaphore("a_s") as a_s,
        nc.semaphore("o_s") as o_s,
        nc.Block() as block,
    ):
        hoist = []

        @block.sync
        def _(sync):
            hoist.append(
                ("SP", sync.dma_start(idx_t[:, :], idx[:, :]).then_inc(i_s, 16))
            )

        @block.gpsimd
        def _(g):
            g.wait_ge(i_s, 16)
            g.dma_gather(
                out_ap=c_t[:, :, :],
                in_ap=cen[:, :],
                idxs_ap=idx_t[:, :],
                num_idxs=SHARD,
                num_idxs_reg=SHARD,
                elem_size=D,
            ).then_inc(g_s, 16)

        @block.vector
        def _(v):
            v.wait_ge(x_s, 16)
            v.wait_ge(g_s, 16)
            v.tensor_sub(diff[:, :, :], x_t[:, :, :], c_t[:, :, :]).then_inc(
                v_s, 1
            )

        @block.scalar
        def _(s):
            hoist.append(
                ("Activation", s.dma_start(x_t[:, :, :], x[:, :, :]).then_inc(x_s, 16))
            )
            # dummy op forces the Square act-table load at ACT program
            # start, off the critical path
            s.activation(
                warm[:, :], ones_bf[:1, :], mybir.ActivationFunctionType.Square
            )
            s.wait_ge(v_s, 1)
            s.activation(
                sq[:, :, :],
                diff[:, :, :],
                mybir.ActivationFunctionType.Square,
                accum_out=d[:, 0:1],
            ).then_inc(a_s, 1)
            s.wait_ge(a_s, 1)
            hoist.append(
                (None, s.dma_start(out[:, :], d[:, :]).then_inc(o_s, 16))
            )

    # Hoist the input DMAs into the entry block, after each engine's
    # barrier-arrival DRAIN but before its release EVSEM
    # ("barrier_<Eng>_*"): the DMA then issues during the const-init
    # barrier window and its ~2.8us completion chain overlaps it.
    entry = nc.m.functions[0].blocks[0]
    for eng_name, handle in hoist:
        if eng_name is None:
            continue
        inst = handle.ins
        for blk in nc.m.functions[0].blocks:
            if inst in blk.instructions:
                blk.instructions.remove(inst)
                break
        barrier_idx = next(
            i
            for i, ins in enumerate(entry.instructions)
            if ins.name.startswith(f"barrier_{eng_name}")
        )
        entry.instructions.insert(barrier_idx, inst)

    # End-block restructure for ACT (it issues the out DMA): its DRAIN
    # blocks on the out-DMA completion and carries the barrier-arrival
    # inc, so all engines' teardown would wait for it. Move the arrival
    # inc to a fresh EVSEM placed before the out-DMA issue and run the
    # drain after the barrier passes.
    end_blk = nc.m.functions[0].blocks[-1]
    act_drain = next(
        ins
        for ins in end_blk.instructions
        if isinstance(ins, mybir.InstDrain)
        and ins.engine == mybir.EngineType.Activation
    )
    act_evsem = next(
        ins
        for ins in end_blk.instructions
        if ins.name.startswith("barrier_Activation")
    )
    arrive = mybir.InstEventSemaphore(
        name=nc.get_next_instruction_name(), ins=[], outs=[]
    )
    arrive.engine = mybir.EngineType.Activation
    arrive.sync_info = act_drain.sync_info
    act_drain.sync_info = None
    nc.register_instruction(arrive)
    end_blk.instructions.remove(act_drain)
    ei = end_blk.instructions.index(act_evsem)
    end_blk.instructions.insert(ei + 1, act_drain)
    out_inst = hoist[-1][1].ins
    body_blk = next(
        blk
        for blk in nc.m.functions[0].blocks
        if out_inst in blk.instructions
    )
    oi = body_blk.instructions.index(out_inst)
    body_blk.instructions.insert(oi, arrive)

    nc.compile()
    return nc


def _get_nc():
    if "nc" not in _CACHE:
        _CACHE["nc"] = build_nc()
    return _CACHE["nc"]


def make_in_maps(x, labels, centers):
    bf16 = ml_dtypes.bfloat16
    x = np.ascontiguousarray(np.asarray(x), dtype=np.float32).astype(bf16)
    labels = np.ascontiguousarray(np.asarray(labels)).astype(np.int16)
    centers = np.ascontiguousarray(
        np.asarray(centers), dtype=np.float32
    ).astype(bf16)
    in_maps = []
    for i in range(M):
        sl = slice(i * SHARD, (i + 1) * SHARD)
        ls = labels[sl]
        # idx[r, c] = labels[c*16 + r], rows replicated to 128 partitions
        idx16 = np.ascontiguousarray(ls.reshape(NIC, 16).T)  # [16, NIC]
        idx_full = np.tile(idx16, (P // 16, 1))              # [128, NIC]
        in_maps.append(
            {
                # x_t[p, n, :] = x[n*128 + p, :]
                "x": np.ascontiguousarray(
                    x[sl].reshape(NT, P, D).transpose(1, 0, 2)
                ),
                "idx": idx_full,
                "cen": centers,
            }
        )
    return in_maps


def finish(partials):
    total = float(np.sum(np.asarray(partials, dtype=np.float64)))
    total += B * (C - 1) * 1e-12  # masked-out entries clamp to 1e-12
    return np.float32(total / B)


def kernel(x, labels, centers):
    from concourse import bass_utils

    nc = _get_nc()
    res = bass_utils.run_bass_kernel_spmd(
        nc, make_in_maps(x, labels, centers), list(range(M))
    )
    return finish([np.asarray(r["out"], dtype=np.float64) for r in res.results])
